# revision 19
# baseline (speedup 1.0000x reference)
"""Causal attention (single head, d=1024) on 8 trn2 NeuronCores — v5.

out = softmax(mask(QK^T)/sqrt(1024)) @ V with mask j <= i+1,
x[4,2048,1024], Wq/Wk/Wv[1024,1024] fp32.

Sharding: 2 cores per batch; core handles 8 of 16 row-blocks
(A = {g%4 in {0,3}}, B = {g%4 in {1,2}} — consecutive (2p,2p+1) pairs
split one-each, balancing causal work). The host permutes x rows to
[own blocks | other blocks], transposes, and pre-splits every operand
into bf16 hi/lo pairs, so the kernel does no transposes and no hi/lo
splitting of inputs; the program is identical on all cores (SPMD),
with all per-core differences in the data.

Algorithm: scores = x Wq (x Wk)^T = x G x^T with G = Wq Wk^T
[1024,1024]. G is batch-independent, so it is computed 8-way split:
each core computes 128 rows (its host-sliced Wq^T columns) right at
kernel start and an AllGather over all 8 cores assembles the full G
while phase V runs; T = x_own G then plays Q's role: S = T x^T.
Projections Q,K never materialize.

Precision: scores need ~2^-16 relative accuracy; G, T and S stages all
use 3-term split-bf16 matmuls (hi/lo decomposition). V and P (attention
weights) are single-term bf16 (~2^-9, ample under the 2e-2 gate).

Causality: per own row-block l the score strip is exactly
[own blocks 0..min(l+1,7)] + [other blocks 0..l] (union over the two
roles; 79 of 128 possible 128-col blocks), with additive -1e30 mask
windows (host data) on the <=3 non-trivial blocks. Attention row-blocks
run in descending-width order so the serial softmax/PV tail is short.

P^T for the P@V matmul is produced by DMA xbar transposes (off the
tensor engine).
"""

import numpy as np
import ml_dtypes

import concourse.bass as bass
import concourse.mybir as mybir
import concourse.tile as tile
from concourse import bacc
from concourse.bass_utils import run_bass_kernel_spmd
from contextlib import ExitStack

B, S, D, DA = 4, 2048, 1024, 1024
NCORES = 8
F32 = mybir.dt.float32
BF16 = mybir.dt.bfloat16

ABLK = [0, 3, 4, 7, 8, 11, 12, 15]
BBLK = [1, 2, 5, 6, 9, 10, 13, 14]
NEG = -1e30

OWN_W = [min(l + 2, 8) for l in range(8)]  # own-run width (128-blocks)
OTH_W = [l + 1 for l in range(8)]          # other-run width


def _strip_positions(l):
    """strip block index -> permuted 128-block position (= V tile index)."""
    return list(range(OWN_W[l])) + [8 + k for k in range(OTH_W[l])]


def _chunks(l):
    """S-matmul chunks: (xt_col_start, strip_col_start, width<=512)."""
    out = []
    ow = OWN_W[l] * 128
    for st in range(0, ow, 512):
        out.append((st, st, min(512, ow - st)))
    tw = OTH_W[l] * 128
    for st in range(0, tw, 512):
        out.append((1024 + st, ow + st, min(512, tw - st)))
    return out


_CACHE = {}


def _build():
    if "nc" in _CACHE:
        return _CACHE["nc"]

    nc = bacc.Bacc()
    xth_d = nc.dram_tensor("xth", [D, S], BF16, kind="ExternalInput")
    xtl_d = nc.dram_tensor("xtl", [D, S], BF16, kind="ExternalInput")
    wqh_d = nc.dram_tensor("wqh", [DA, D], BF16, kind="ExternalInput")
    wql_d = nc.dram_tensor("wql", [DA, D], BF16, kind="ExternalInput")
    wkh_d = nc.dram_tensor("wkh", [DA, D], BF16, kind="ExternalInput")
    wkl_d = nc.dram_tensor("wkl", [DA, D], BF16, kind="ExternalInput")
    wvb_d = nc.dram_tensor("wvb", [D, DA], BF16, kind="ExternalInput")
    mka_d = nc.dram_tensor("mka", [1024, 256], BF16, kind="ExternalInput")
    mkb_d = nc.dram_tensor("mkb", [1024, 128], BF16, kind="ExternalInput")
    out_d = nc.dram_tensor("out", [1024, DA], F32, kind="ExternalOutput")

    with tile.TileContext(nc) as tc, ExitStack() as stack:
        # long-lived residents
        xpool = stack.enter_context(tc.tile_pool(name="xres", bufs=1))
        # xth[g][p, dp*512 + c] = bf16(x^T[dp*128+p, g*512+c]); xtl the residual
        xth = [xpool.tile([128, 4096], BF16, name=f"xth{g}", tag=f"xth{g}") for g in range(4)]
        xtl = [xpool.tile([128, 4096], BF16, name=f"xtl{g}", tag=f"xtl{g}") for g in range(4)]
        vpool = stack.enter_context(tc.tile_pool(name="vres", bufs=1))
        V = [vpool.tile([128, DA], BF16, name=f"v{p}", tag=f"v{p}") for p in range(16)]
        gpool = stack.enter_context(tc.tile_pool(name="gres", bufs=1))
        # G[d1, d2] tiles [d1-part][128, 1024] (hi/lo)
        Gh = [gpool.tile([128, 1024], BF16, name=f"gh{d}", tag=f"gh{d}") for d in range(8)]
        Gl = [gpool.tile([128, 1024], BF16, name=f"gl{d}", tag=f"gl{d}") for d in range(8)]

        with tc.tile_pool(name="wvp", bufs=1) as pwv:
            # Queue plan (per-engine order = execution order):
            #  sync:   xth-g0, wk-hi, xtl-g0, xtl-g1, gin ship, G-lo readback, (outs via gpsimd)
            #  scalar: wqs, wk-lo, xth-g1, G-hi readback, masks/PT later
            #  gpsimd: wv, xth-g2, xth-g3, collective, xtl-g2, xtl-g3, out stores
            # half-width wv tiles (reloaded once per DA-half) keep the W
            # pool + wv concurrently under the SBUF budget
            wvb = [pwv.tile([128, 512], BF16, name=f"wvb{d}", tag=f"wvb{d}") for d in range(8)]

            with (
                tc.tile_pool(name="wqk", bufs=1) as pw,
                tc.tile_pool(name="psG", bufs=3, space="PSUM") as psG,
            ):
                wkth = [pw.tile([128, D], BF16, name=f"wkth{a}", tag=f"wkth{a}") for a in range(8)]
                wktl = [pw.tile([128, D], BF16, name=f"wktl{a}", tag=f"wktl{a}") for a in range(8)]
                wqth = [pw.tile([128, D], BF16, name=f"wqth{a}", tag=f"wqth{a}") for a in range(8)]
                wqtl = [pw.tile([128, D], BF16, name=f"wqtl{a}", tag=f"wqtl{a}") for a in range(8)]
                # x-hi first on every queue so phase V is never starved
                for dp in range(8):
                    nc.sync.dma_start(
                        xth[0][:, dp * 512 : (dp + 1) * 512],
                        xth_d[dp * 128 : (dp + 1) * 128, 0:512],
                    )
                for dp in range(8):
                    nc.scalar.dma_start(
                        xth[1][:, dp * 512 : (dp + 1) * 512],
                        xth_d[dp * 128 : (dp + 1) * 128, 512:1024],
                    )
                for g in (2, 3):
                    for dp in range(8):
                        nc.gpsimd.dma_start(
                            xth[g][:, dp * 512 : (dp + 1) * 512],
                            xth_d[dp * 128 : (dp + 1) * 128, g * 512 : (g + 1) * 512],
                        )
                # W hi on sync, lo on scalar (per-ap interleave = G consumption order)
                for ap in range(8):
                    rsl = slice(ap * 128, (ap + 1) * 128)
                    nc.sync.dma_start(wqth[ap][:], wqh_d[rsl, :])
                    nc.sync.dma_start(wkth[ap][:], wkh_d[rsl, :])
                    nc.scalar.dma_start(wqtl[ap][:], wql_d[rsl, :])
                    nc.scalar.dma_start(wktl[ap][:], wkl_d[rsl, :])
                # x-lo: T-critical groups on sync, rest on gpsimd
                for g in (0, 1):
                    for dp in range(8):
                        nc.sync.dma_start(
                            xtl[g][:, dp * 512 : (dp + 1) * 512],
                            xtl_d[dp * 128 : (dp + 1) * 128, g * 512 : (g + 1) * 512],
                        )
                for g in (2, 3):
                    for dp in range(8):
                        nc.gpsimd.dma_start(
                            xtl[g][:, dp * 512 : (dp + 1) * 512],
                            xtl_d[dp * 128 : (dp + 1) * 128, g * 512 : (g + 1) * 512],
                        )

                with tc.tile_pool(name="psV", bufs=4, space="PSUM") as psV:
                    for half in range(2):
                        for dp in range(8):
                            nc.gpsimd.dma_start(
                                wvb[dp][:],
                                wvb_d[dp * 128 : (dp + 1) * 128, half * 512 : (half + 1) * 512],
                            )
                        for g in range(4):
                            for q in range(4):
                                p = g * 4 + q
                                ps = psV.tile([128, 512], F32, name="psv", tag="psv")
                                for dp in range(8):
                                    nc.tensor.matmul(
                                        ps[:],
                                        xth[g][:, dp * 512 + q * 128 : dp * 512 + (q + 1) * 128],
                                        wvb[dp][:],
                                        start=(dp == 0),
                                        stop=(dp == 7),
                                    )
                                nc.vector.tensor_copy(V[p][:, half * 512 : (half + 1) * 512], ps[:])

                # ---- Phase G: full G = Wq Wk^T (3-term split-bf16) --------
                for d1 in range(8):
                    d1sl = slice(d1 * 128, (d1 + 1) * 128)
                    for half in range(2):
                        hsl = slice(half * 512, (half + 1) * 512)
                        ps = psG.tile([128, 512], F32, name="psg", tag="psg")
                        for ap in range(8):
                            nc.tensor.matmul(ps[:], wqth[ap][:, d1sl], wkth[ap][:, hsl], start=(ap == 0), stop=False)
                            nc.tensor.matmul(ps[:], wqth[ap][:, d1sl], wktl[ap][:, hsl], start=False, stop=False)
                            nc.tensor.matmul(ps[:], wqtl[ap][:, d1sl], wkth[ap][:, hsl], start=False, stop=(ap == 7))
                        nc.vector.tensor_copy(Gh[d1][:, hsl], ps[:])
                        nc.vector.tensor_sub(Gl[d1][:, hsl], ps[:], Gh[d1][:, hsl])

        # T^T[d, i] for own rows i (hi/lo): tiles [d-part][128, 1024]
        tpool = stack.enter_context(tc.tile_pool(name="tres", bufs=1))
        Th = [tpool.tile([128, 1024], BF16, name=f"th{d}", tag=f"th{d}") for d in range(8)]
        Tl = [tpool.tile([128, 1024], BF16, name=f"tl{d}", tag=f"tl{d}") for d in range(8)]

        # ---- Phase T: T^T = (x_own G)^T, own rows = groups 0,1 ------------
        with tc.tile_pool(name="psT", bufs=3, space="PSUM") as psT:
            for dpp in range(8):
                dsl = slice(dpp * 128, (dpp + 1) * 128)
                for half in range(2):
                    hsl = slice(half * 512, (half + 1) * 512)
                    ps = psT.tile([128, 512], F32, name="pst", tag="pst")
                    for dp in range(8):
                        rh = xth[half][:, dp * 512 : (dp + 1) * 512]
                        rl = xtl[half][:, dp * 512 : (dp + 1) * 512]
                        nc.tensor.matmul(ps[:], Gh[dp][:, dsl], rh, start=(dp == 0), stop=False)
                        nc.tensor.matmul(ps[:], Gh[dp][:, dsl], rl, start=False, stop=False)
                        nc.tensor.matmul(ps[:], Gl[dp][:, dsl], rh, start=False, stop=(dp == 7))
                    nc.vector.tensor_copy(Th[dpp][:, hsl], ps[:])
                    nc.vector.tensor_sub(Tl[dpp][:, hsl], ps[:], Th[dpp][:, hsl])

        # ---- Phase attn: per own row-block l, descending strip width ------
        with (
            tc.tile_pool(name="pa", bufs=2) as pa,
            tc.tile_pool(name="pa1", bufs=3) as pa1,
            tc.tile_pool(name="psS", bufs=3, space="PSUM") as psS,
            tc.tile_pool(name="psO", bufs=2, space="PSUM") as psO,
        ):
            state = {}

            def emit_S(l):
                W = OWN_W[l] + OTH_W[l]
                S_sb = pa.tile([128, 2048], F32, name="S_sb", tag="S")
                for (xc0, sc0, w) in _chunks(l):
                    g, off = divmod(xc0, 512)
                    ps = psS.tile([128, 512], F32, name="ps", tag="ps")
                    for dp in range(8):
                        lh = Th[dp][:, l * 128 : (l + 1) * 128]
                        ll = Tl[dp][:, l * 128 : (l + 1) * 128]
                        rh = xth[g][:, dp * 512 + off : dp * 512 + off + w]
                        rl = xtl[g][:, dp * 512 + off : dp * 512 + off + w]
                        nc.tensor.matmul(ps[:, :w], lh, rh, start=(dp == 0), stop=False)
                        nc.tensor.matmul(ps[:, :w], lh, rl, start=False, stop=False)
                        nc.tensor.matmul(ps[:, :w], ll, rh, start=False, stop=(dp == 7))
                    nc.vector.tensor_copy(S_sb[:, sc0 : sc0 + w], ps[:, :w])
                mka = pa1.tile([128, 256], BF16, name="mka", tag="mka")
                nc.scalar.dma_start(mka[:], mka_d[l * 128 : (l + 1) * 128, :])
                w1 = slice(l * 128, (l + 2) * 128)
                nc.vector.tensor_add(S_sb[:, w1], S_sb[:, w1], mka[:])
                mkb = pa1.tile([128, 128], BF16, name="mkb", tag="mkb")
                nc.scalar.dma_start(mkb[:], mkb_d[l * 128 : (l + 1) * 128, :])
                w2 = slice((W - 1) * 128, W * 128)
                nc.vector.tensor_add(S_sb[:, w2], S_sb[:, w2], mkb[:])

                mx = pa1.tile([128, 1], F32, name="mx", tag="mx")
                nc.vector.reduce_max(mx[:], S_sb[:, : W * 128], axis=mybir.AxisListType.X)
                negb = pa1.tile([128, 1], F32, name="negb", tag="negb")
                nc.vector.tensor_scalar_mul(negb[:], mx[:], -1.0 / 32.0)
                P_sb = pa.tile([128, 2048], BF16, name="P_sb", tag="P")
                rs = pa1.tile([128, 1], F32, name="rs", tag="rs")
                nc.scalar.activation(
                    P_sb[:, : W * 128],
                    S_sb[:, : W * 128],
                    mybir.ActivationFunctionType.Exp,
                    bias=negb[:],
                    scale=1.0 / 32.0,
                    accum_out=rs[:],
                )
                PT = pa.tile([128, 2048], BF16, name="PT", tag="PT")
                eng = [nc.sync, nc.scalar]
                for b in range(W):
                    bsl = slice(b * 128, (b + 1) * 128)
                    eng[b % 2].dma_start_transpose(PT[:, bsl], P_sb[:, bsl])
                state[l] = (W, PT, rs)

            def emit_PV(l):
                W, PT, rs = state.pop(l)
                pos = _strip_positions(l)
                oacc = [psO.tile([128, 512], F32, name=f"oacc{h}", tag=f"oacc{h}") for h in range(2)]
                for b in range(W):
                    vj = pos[b]
                    bsl = slice(b * 128, (b + 1) * 128)
                    for half in range(2):
                        nc.tensor.matmul(
                            oacc[half][:],
                            PT[:, bsl],
                            V[vj][:, half * 512 : (half + 1) * 512],
                            start=(b == 0),
                            stop=(b == W - 1),
                        )
                rec = pa1.tile([128, 1], F32, name="rec", tag="rec")
                nc.vector.reciprocal(rec[:], rs[:])
                for half in range(2):
                    o_sb = pa1.tile([128, 512], F32, name=f"o{half}", tag=f"o{half}")
                    nc.vector.tensor_scalar_mul(o_sb[:], oacc[half][:], rec[:])
                    nc.gpsimd.dma_start(
                        out_d[l * 128 : (l + 1) * 128, half * 512 : (half + 1) * 512],
                        o_sb[:],
                    )

            order = list(range(7, -1, -1))  # descending strip width
            for i, l in enumerate(order):
                emit_S(l)
                if i >= 1:
                    emit_PV(order[i - 1])
            emit_PV(order[-1])

    nc.compile()
    _CACHE["nc"] = nc
    return nc


def _split_bf16(a):
    hi = a.astype(ml_dtypes.bfloat16)
    lo = (a - hi.astype(np.float32)).astype(ml_dtypes.bfloat16)
    return np.ascontiguousarray(hi), np.ascontiguousarray(lo)


def _core_inputs(x, Wq, Wk, Wv, c):
    b = c // 2
    my = ABLK if c % 2 == 0 else BBLK
    oth = BBLK if c % 2 == 0 else ABLK
    permrows = np.concatenate([np.arange(g * 128, (g + 1) * 128) for g in my + oth])
    xt = np.ascontiguousarray(x[b][permrows].T)
    xth, xtl = _split_bf16(xt)
    wqh_f, wql_f = _split_bf16(np.ascontiguousarray(Wq.T))
    wqh, wql = wqh_f, wql_f
    wkh, wkl = _split_bf16(np.ascontiguousarray(Wk.T))

    mka = np.zeros((1024, 256), dtype=ml_dtypes.bfloat16)
    mkb = np.zeros((1024, 128), dtype=ml_dtypes.bfloat16)
    for l in range(8):
        gi = my[l] * 128 + np.arange(128)
        strip = [my[k] for k in range(OWN_W[l])] + [oth[k] for k in range(OTH_W[l])]
        W = len(strip)
        for t, blk in enumerate((strip[l], strip[l + 1])):
            gj = blk * 128 + np.arange(128)
            mka[l * 128 : (l + 1) * 128, t * 128 : (t + 1) * 128] = np.where(
                gj[None, :] <= gi[:, None] + 1, 0.0, NEG
            )
        blk = strip[W - 1]
        gj = blk * 128 + np.arange(128)
        mkb[l * 128 : (l + 1) * 128, :] = np.where(gj[None, :] <= gi[:, None] + 1, 0.0, NEG)
        for p2, blk2 in enumerate(strip):
            if p2 in (l, l + 1, W - 1):
                continue
            assert blk2 * 128 + 127 <= my[l] * 128 + 1, (l, p2, blk2)

    return {
        "xth": xth,
        "xtl": xtl,
        "wqh": wqh,
        "wql": wql,
        "wkh": wkh,
        "wkl": wkl,
        "wvb": Wv.astype(ml_dtypes.bfloat16),
        "mka": mka,
        "mkb": mkb,
    }, (b, my)


def kernel(x, Wq, Wk, Wv):
    x = np.ascontiguousarray(np.asarray(x, dtype=np.float32))
    Wq = np.ascontiguousarray(np.asarray(Wq, dtype=np.float32))
    Wk = np.ascontiguousarray(np.asarray(Wk, dtype=np.float32))
    Wv = np.ascontiguousarray(np.asarray(Wv, dtype=np.float32))

    nc = _build()

    in_maps = []
    metas = []
    for c in range(NCORES):
        m, meta = _core_inputs(x, Wq, Wk, Wv, c)
        in_maps.append(m)
        metas.append(meta)

    res = run_bass_kernel_spmd(nc, in_maps, list(range(NCORES)))

    out = np.empty((B, S, DA), dtype=np.float32)
    for c in range(NCORES):
        b, my = metas[c]
        o = res.results[c]["out"]
        for l, g in enumerate(my):
            out[b, g * 128 : (g + 1) * 128] = o[l * 128 : (l + 1) * 128]
    return out


# revision 20
# speedup vs baseline: 1.0109x; 1.0109x over previous
"""Causal attention (single head, d=1024) on 8 trn2 NeuronCores — v5.

out = softmax(mask(QK^T)/sqrt(1024)) @ V with mask j <= i+1,
x[4,2048,1024], Wq/Wk/Wv[1024,1024] fp32.

Sharding: 2 cores per batch; core handles 8 of 16 row-blocks
(A = {g%4 in {0,3}}, B = {g%4 in {1,2}} — consecutive (2p,2p+1) pairs
split one-each, balancing causal work). The host permutes x rows to
[own blocks | other blocks], transposes, and pre-splits every operand
into bf16 hi/lo pairs, so the kernel does no transposes and no hi/lo
splitting of inputs; the program is identical on all cores (SPMD),
with all per-core differences in the data.

Algorithm: scores = x Wq (x Wk)^T = x G x^T with G = Wq Wk^T
[1024,1024]. G is batch-independent, so it is computed 8-way split:
each core computes 128 rows (its host-sliced Wq^T columns) right at
kernel start and an AllGather over all 8 cores assembles the full G
while phase V runs; T = x_own G then plays Q's role: S = T x^T.
Projections Q,K never materialize.

Precision: scores need ~2^-16 relative accuracy; G, T and S stages all
use 3-term split-bf16 matmuls (hi/lo decomposition). V and P (attention
weights) are single-term bf16 (~2^-9, ample under the 2e-2 gate).

Causality: per own row-block l the score strip is exactly
[own blocks 0..min(l+1,7)] + [other blocks 0..l] (union over the two
roles; 79 of 128 possible 128-col blocks), with additive -1e30 mask
windows (host data) on the <=3 non-trivial blocks. Attention row-blocks
run in descending-width order so the serial softmax/PV tail is short.

P^T for the P@V matmul is produced by DMA xbar transposes (off the
tensor engine).
"""

import numpy as np
import ml_dtypes

import concourse.bass as bass
import concourse.mybir as mybir
import concourse.tile as tile
from concourse import bacc
from concourse.bass_utils import run_bass_kernel_spmd
from contextlib import ExitStack

B, S, D, DA = 4, 2048, 1024, 1024
NCORES = 8
F32 = mybir.dt.float32
BF16 = mybir.dt.bfloat16

ABLK = [0, 3, 4, 7, 8, 11, 12, 15]
BBLK = [1, 2, 5, 6, 9, 10, 13, 14]
NEG = -1e30

OWN_W = [min(l + 2, 8) for l in range(8)]  # own-run width (128-blocks)
OTH_W = [l + 1 for l in range(8)]          # other-run width


def _strip_positions(l):
    """strip block index -> permuted 128-block position (= V tile index)."""
    return list(range(OWN_W[l])) + [8 + k for k in range(OTH_W[l])]


def _chunks(l):
    """S-matmul chunks: (xt_col_start, strip_col_start, width<=512)."""
    out = []
    ow = OWN_W[l] * 128
    for st in range(0, ow, 512):
        out.append((st, st, min(512, ow - st)))
    tw = OTH_W[l] * 128
    for st in range(0, tw, 512):
        out.append((1024 + st, ow + st, min(512, tw - st)))
    return out


_CACHE = {}


def _build():
    if "nc" in _CACHE:
        return _CACHE["nc"]

    nc = bacc.Bacc()
    xth_d = nc.dram_tensor("xth", [D, S], BF16, kind="ExternalInput")
    xtl_d = nc.dram_tensor("xtl", [D, S], BF16, kind="ExternalInput")
    wqh_d = nc.dram_tensor("wqh", [DA, D], BF16, kind="ExternalInput")
    wql_d = nc.dram_tensor("wql", [DA, D], BF16, kind="ExternalInput")
    wkh_d = nc.dram_tensor("wkh", [DA, D], BF16, kind="ExternalInput")
    wkl_d = nc.dram_tensor("wkl", [DA, D], BF16, kind="ExternalInput")
    wvb_d = nc.dram_tensor("wvb", [D, DA], BF16, kind="ExternalInput")
    mka_d = nc.dram_tensor("mka", [1024, 256], BF16, kind="ExternalInput")
    mkb_d = nc.dram_tensor("mkb", [1024, 128], BF16, kind="ExternalInput")
    out_d = nc.dram_tensor("out", [1024, DA], F32, kind="ExternalOutput")

    with tile.TileContext(nc) as tc, ExitStack() as stack:
        # long-lived residents
        xpool = stack.enter_context(tc.tile_pool(name="xres", bufs=1))
        # xth[g][p, dp*512 + c] = bf16(x^T[dp*128+p, g*512+c]); xtl the residual
        xth = [xpool.tile([128, 4096], BF16, name=f"xth{g}", tag=f"xth{g}") for g in range(4)]
        xtl = [xpool.tile([128, 4096], BF16, name=f"xtl{g}", tag=f"xtl{g}") for g in range(4)]
        vpool = stack.enter_context(tc.tile_pool(name="vres", bufs=1))
        V = [vpool.tile([128, DA], BF16, name=f"v{p}", tag=f"v{p}") for p in range(16)]
        gpool = stack.enter_context(tc.tile_pool(name="gres", bufs=1))
        # G[d1, d2] tiles [d1-part][128, 1024] (hi/lo)
        Gh = [gpool.tile([128, 1024], BF16, name=f"gh{d}", tag=f"gh{d}") for d in range(8)]
        Gl = [gpool.tile([128, 1024], BF16, name=f"gl{d}", tag=f"gl{d}") for d in range(8)]

        with tc.tile_pool(name="wvp", bufs=1) as pwv:
            # Queue plan (per-engine order = execution order):
            #  sync:   xth-g0, wk-hi, xtl-g0, xtl-g1, gin ship, G-lo readback, (outs via gpsimd)
            #  scalar: wqs, wk-lo, xth-g1, G-hi readback, masks/PT later
            #  gpsimd: wv, xth-g2, xth-g3, collective, xtl-g2, xtl-g3, out stores
            # half-width wv tiles (reloaded once per DA-half) keep the W
            # pool + wv concurrently under the SBUF budget
            wvb = [pwv.tile([128, 512], BF16, name=f"wvb{d}", tag=f"wvb{d}") for d in range(8)]
            for dp in range(8):  # half 0 — first on gpsimd so V starts at ~3us
                nc.gpsimd.dma_start(wvb[dp][:], wvb_d[dp * 128 : (dp + 1) * 128, 0:512])

            with (
                tc.tile_pool(name="wqk", bufs=1) as pw,
                tc.tile_pool(name="psG", bufs=3, space="PSUM") as psG,
            ):
                wkth = [pw.tile([128, D], BF16, name=f"wkth{a}", tag=f"wkth{a}") for a in range(8)]
                wktl = [pw.tile([128, D], BF16, name=f"wktl{a}", tag=f"wktl{a}") for a in range(8)]
                wqth = [pw.tile([128, D], BF16, name=f"wqth{a}", tag=f"wqth{a}") for a in range(8)]
                wqtl = [pw.tile([128, D], BF16, name=f"wqtl{a}", tag=f"wqtl{a}") for a in range(8)]
                # x-hi first on every queue so phase V is never starved
                for dp in range(8):
                    nc.sync.dma_start(
                        xth[0][:, dp * 512 : (dp + 1) * 512],
                        xth_d[dp * 128 : (dp + 1) * 128, 0:512],
                    )
                for dp in range(8):
                    nc.scalar.dma_start(
                        xth[1][:, dp * 512 : (dp + 1) * 512],
                        xth_d[dp * 128 : (dp + 1) * 128, 512:1024],
                    )
                for g in (2, 3):
                    for dp in range(8):
                        nc.gpsimd.dma_start(
                            xth[g][:, dp * 512 : (dp + 1) * 512],
                            xth_d[dp * 128 : (dp + 1) * 128, g * 512 : (g + 1) * 512],
                        )
                # W hi on sync, lo on scalar (per-ap interleave = G consumption order)
                for ap in range(8):
                    rsl = slice(ap * 128, (ap + 1) * 128)
                    nc.sync.dma_start(wqth[ap][:], wqh_d[rsl, :])
                    nc.sync.dma_start(wkth[ap][:], wkh_d[rsl, :])
                    nc.scalar.dma_start(wqtl[ap][:], wql_d[rsl, :])
                    nc.scalar.dma_start(wktl[ap][:], wkl_d[rsl, :])
                # x-lo: T-critical groups on sync, rest on gpsimd
                for g in (0, 1):
                    for dp in range(8):
                        nc.sync.dma_start(
                            xtl[g][:, dp * 512 : (dp + 1) * 512],
                            xtl_d[dp * 128 : (dp + 1) * 128, g * 512 : (g + 1) * 512],
                        )
                for g in (2, 3):
                    for dp in range(8):
                        nc.gpsimd.dma_start(
                            xtl[g][:, dp * 512 : (dp + 1) * 512],
                            xtl_d[dp * 128 : (dp + 1) * 128, g * 512 : (g + 1) * 512],
                        )

                with tc.tile_pool(name="psV", bufs=4, space="PSUM") as psV:
                    for half in range(2):
                        if half == 1:
                            for dp in range(8):
                                nc.scalar.dma_start(
                                    wvb[dp][:],
                                    wvb_d[dp * 128 : (dp + 1) * 128, 512:1024],
                                )
                        for g in range(4):
                            for q in range(4):
                                p = g * 4 + q
                                ps = psV.tile([128, 512], F32, name="psv", tag="psv")
                                for dp in range(8):
                                    nc.tensor.matmul(
                                        ps[:],
                                        xth[g][:, dp * 512 + q * 128 : dp * 512 + (q + 1) * 128],
                                        wvb[dp][:],
                                        start=(dp == 0),
                                        stop=(dp == 7),
                                    )
                                nc.vector.tensor_copy(V[p][:, half * 512 : (half + 1) * 512], ps[:])

                # ---- Phase G: full G = Wq Wk^T (3-term split-bf16) --------
                for d1 in range(8):
                    d1sl = slice(d1 * 128, (d1 + 1) * 128)
                    for half in range(2):
                        hsl = slice(half * 512, (half + 1) * 512)
                        ps = psG.tile([128, 512], F32, name="psg", tag="psg")
                        for ap in range(8):
                            nc.tensor.matmul(ps[:], wqth[ap][:, d1sl], wkth[ap][:, hsl], start=(ap == 0), stop=False)
                            nc.tensor.matmul(ps[:], wqth[ap][:, d1sl], wktl[ap][:, hsl], start=False, stop=False)
                            nc.tensor.matmul(ps[:], wqtl[ap][:, d1sl], wkth[ap][:, hsl], start=False, stop=(ap == 7))
                        nc.vector.tensor_copy(Gh[d1][:, hsl], ps[:])
                        nc.vector.tensor_sub(Gl[d1][:, hsl], ps[:], Gh[d1][:, hsl])

        # T^T[d, i] for own rows i (hi/lo): tiles [d-part][128, 1024]
        tpool = stack.enter_context(tc.tile_pool(name="tres", bufs=1))
        Th = [tpool.tile([128, 1024], BF16, name=f"th{d}", tag=f"th{d}") for d in range(8)]
        Tl = [tpool.tile([128, 1024], BF16, name=f"tl{d}", tag=f"tl{d}") for d in range(8)]

        # ---- Phase T: T^T = (x_own G)^T, own rows = groups 0,1 ------------
        with tc.tile_pool(name="psT", bufs=3, space="PSUM") as psT:
            for dpp in range(8):
                dsl = slice(dpp * 128, (dpp + 1) * 128)
                for half in range(2):
                    hsl = slice(half * 512, (half + 1) * 512)
                    ps = psT.tile([128, 512], F32, name="pst", tag="pst")
                    for dp in range(8):
                        rh = xth[half][:, dp * 512 : (dp + 1) * 512]
                        rl = xtl[half][:, dp * 512 : (dp + 1) * 512]
                        nc.tensor.matmul(ps[:], Gh[dp][:, dsl], rh, start=(dp == 0), stop=False)
                        nc.tensor.matmul(ps[:], Gh[dp][:, dsl], rl, start=False, stop=False)
                        nc.tensor.matmul(ps[:], Gl[dp][:, dsl], rh, start=False, stop=(dp == 7))
                    nc.vector.tensor_copy(Th[dpp][:, hsl], ps[:])
                    nc.vector.tensor_sub(Tl[dpp][:, hsl], ps[:], Th[dpp][:, hsl])

        # ---- Phase attn: per own row-block l, descending strip width ------
        with (
            tc.tile_pool(name="pa", bufs=2) as pa,
            tc.tile_pool(name="pa1", bufs=3) as pa1,
            tc.tile_pool(name="psS", bufs=3, space="PSUM") as psS,
            tc.tile_pool(name="psO", bufs=2, space="PSUM") as psO,
        ):
            state = {}

            def emit_S(l):
                W = OWN_W[l] + OTH_W[l]
                S_sb = pa.tile([128, 2048], F32, name="S_sb", tag="S")
                for (xc0, sc0, w) in _chunks(l):
                    g, off = divmod(xc0, 512)
                    ps = psS.tile([128, 512], F32, name="ps", tag="ps")
                    for dp in range(8):
                        lh = Th[dp][:, l * 128 : (l + 1) * 128]
                        ll = Tl[dp][:, l * 128 : (l + 1) * 128]
                        rh = xth[g][:, dp * 512 + off : dp * 512 + off + w]
                        rl = xtl[g][:, dp * 512 + off : dp * 512 + off + w]
                        nc.tensor.matmul(ps[:, :w], lh, rh, start=(dp == 0), stop=False)
                        nc.tensor.matmul(ps[:, :w], lh, rl, start=False, stop=False)
                        nc.tensor.matmul(ps[:, :w], ll, rh, start=False, stop=(dp == 7))
                    nc.vector.tensor_copy(S_sb[:, sc0 : sc0 + w], ps[:, :w])
                mka = pa1.tile([128, 256], BF16, name="mka", tag="mka")
                nc.scalar.dma_start(mka[:], mka_d[l * 128 : (l + 1) * 128, :])
                w1 = slice(l * 128, (l + 2) * 128)
                nc.vector.tensor_add(S_sb[:, w1], S_sb[:, w1], mka[:])
                mkb = pa1.tile([128, 128], BF16, name="mkb", tag="mkb")
                nc.scalar.dma_start(mkb[:], mkb_d[l * 128 : (l + 1) * 128, :])
                w2 = slice((W - 1) * 128, W * 128)
                nc.vector.tensor_add(S_sb[:, w2], S_sb[:, w2], mkb[:])

                mx = pa1.tile([128, 1], F32, name="mx", tag="mx")
                nc.vector.reduce_max(mx[:], S_sb[:, : W * 128], axis=mybir.AxisListType.X)
                negb = pa1.tile([128, 1], F32, name="negb", tag="negb")
                nc.vector.tensor_scalar_mul(negb[:], mx[:], -1.0 / 32.0)
                P_sb = pa.tile([128, 2048], BF16, name="P_sb", tag="P")
                rs = pa1.tile([128, 1], F32, name="rs", tag="rs")
                nc.scalar.activation(
                    P_sb[:, : W * 128],
                    S_sb[:, : W * 128],
                    mybir.ActivationFunctionType.Exp,
                    bias=negb[:],
                    scale=1.0 / 32.0,
                    accum_out=rs[:],
                )
                PT = pa.tile([128, 2048], BF16, name="PT", tag="PT")
                eng = [nc.sync, nc.scalar]
                for b in range(W):
                    bsl = slice(b * 128, (b + 1) * 128)
                    eng[b % 2].dma_start_transpose(PT[:, bsl], P_sb[:, bsl])
                state[l] = (W, PT, rs)

            def emit_PV(l):
                W, PT, rs = state.pop(l)
                pos = _strip_positions(l)
                oacc = [psO.tile([128, 512], F32, name=f"oacc{h}", tag=f"oacc{h}") for h in range(2)]
                for b in range(W):
                    vj = pos[b]
                    bsl = slice(b * 128, (b + 1) * 128)
                    for half in range(2):
                        nc.tensor.matmul(
                            oacc[half][:],
                            PT[:, bsl],
                            V[vj][:, half * 512 : (half + 1) * 512],
                            start=(b == 0),
                            stop=(b == W - 1),
                        )
                rec = pa1.tile([128, 1], F32, name="rec", tag="rec")
                nc.vector.reciprocal(rec[:], rs[:])
                for half in range(2):
                    o_sb = pa1.tile([128, 512], F32, name=f"o{half}", tag=f"o{half}")
                    nc.vector.tensor_scalar_mul(o_sb[:], oacc[half][:], rec[:])
                    nc.gpsimd.dma_start(
                        out_d[l * 128 : (l + 1) * 128, half * 512 : (half + 1) * 512],
                        o_sb[:],
                    )

            order = list(range(7, -1, -1))  # descending strip width
            for i, l in enumerate(order):
                emit_S(l)
                if i >= 1:
                    emit_PV(order[i - 1])
            emit_PV(order[-1])

    nc.compile()
    _CACHE["nc"] = nc
    return nc


def _split_bf16(a):
    hi = a.astype(ml_dtypes.bfloat16)
    lo = (a - hi.astype(np.float32)).astype(ml_dtypes.bfloat16)
    return np.ascontiguousarray(hi), np.ascontiguousarray(lo)


def _core_inputs(x, Wq, Wk, Wv, c):
    b = c // 2
    my = ABLK if c % 2 == 0 else BBLK
    oth = BBLK if c % 2 == 0 else ABLK
    permrows = np.concatenate([np.arange(g * 128, (g + 1) * 128) for g in my + oth])
    xt = np.ascontiguousarray(x[b][permrows].T)
    xth, xtl = _split_bf16(xt)
    wqh_f, wql_f = _split_bf16(np.ascontiguousarray(Wq.T))
    wqh, wql = wqh_f, wql_f
    wkh, wkl = _split_bf16(np.ascontiguousarray(Wk.T))

    mka = np.zeros((1024, 256), dtype=ml_dtypes.bfloat16)
    mkb = np.zeros((1024, 128), dtype=ml_dtypes.bfloat16)
    for l in range(8):
        gi = my[l] * 128 + np.arange(128)
        strip = [my[k] for k in range(OWN_W[l])] + [oth[k] for k in range(OTH_W[l])]
        W = len(strip)
        for t, blk in enumerate((strip[l], strip[l + 1])):
            gj = blk * 128 + np.arange(128)
            mka[l * 128 : (l + 1) * 128, t * 128 : (t + 1) * 128] = np.where(
                gj[None, :] <= gi[:, None] + 1, 0.0, NEG
            )
        blk = strip[W - 1]
        gj = blk * 128 + np.arange(128)
        mkb[l * 128 : (l + 1) * 128, :] = np.where(gj[None, :] <= gi[:, None] + 1, 0.0, NEG)
        for p2, blk2 in enumerate(strip):
            if p2 in (l, l + 1, W - 1):
                continue
            assert blk2 * 128 + 127 <= my[l] * 128 + 1, (l, p2, blk2)

    return {
        "xth": xth,
        "xtl": xtl,
        "wqh": wqh,
        "wql": wql,
        "wkh": wkh,
        "wkl": wkl,
        "wvb": Wv.astype(ml_dtypes.bfloat16),
        "mka": mka,
        "mkb": mkb,
    }, (b, my)


def kernel(x, Wq, Wk, Wv):
    x = np.ascontiguousarray(np.asarray(x, dtype=np.float32))
    Wq = np.ascontiguousarray(np.asarray(Wq, dtype=np.float32))
    Wk = np.ascontiguousarray(np.asarray(Wk, dtype=np.float32))
    Wv = np.ascontiguousarray(np.asarray(Wv, dtype=np.float32))

    nc = _build()

    in_maps = []
    metas = []
    for c in range(NCORES):
        m, meta = _core_inputs(x, Wq, Wk, Wv, c)
        in_maps.append(m)
        metas.append(meta)

    res = run_bass_kernel_spmd(nc, in_maps, list(range(NCORES)))

    out = np.empty((B, S, DA), dtype=np.float32)
    for c in range(NCORES):
        b, my = metas[c]
        o = res.results[c]["out"]
        for l, g in enumerate(my):
            out[b, g * 128 : (g + 1) * 128] = o[l * 128 : (l + 1) * 128]
    return out


# revision 23
# speedup vs baseline: 1.0508x; 1.0395x over previous
"""Causal attention (single head, d=1024) on 8 trn2 NeuronCores — v3.

out = softmax(mask(QK^T)/sqrt(1024)) @ V with mask j <= i+1,
x[4,2048,1024], Wq/Wk/Wv[1024,1024] fp32.

Sharding: 2 cores per batch; core handles 8 of 16 row-blocks
(A = {g%4 in {0,3}}, B = {g%4 in {1,2}} — consecutive (2p,2p+1) pairs
split one-each, balancing causal work). The host permutes x rows to
[own blocks | other blocks], transposes, and pre-splits every operand
into bf16 hi/lo pairs, so the kernel does no transposes and no hi/lo
splitting of inputs at all; the program is identical on all cores
(SPMD), with per-core content in the data.

Algorithm: scores = x Wq (x Wk)^T = x G x^T with G = Wq Wk^T
[1024,1024]. Computing G (shared across all rows) replaces the
full-sequence K projection; T = x_own G then plays Q's role:
S = T x^T. Projections Q,K never materialize.

Precision: scores need ~2^-16 relative accuracy; G, T and S stages all
use 3-term split-bf16 matmuls (hi/lo decomposition). V and P (attention
weights) are single-term bf16 (~2^-9, ample under the 2e-2 gate).

Causality: per own row-block l the score strip is exactly
[own blocks 0..min(l+1,7)] + [other blocks 0..l] (union over the two
roles; 79 of 128 possible 128-col blocks), with additive -1e30 mask
windows (host data) on the <=3 non-trivial blocks. Attention row-blocks
run in descending-width order so the serial softmax/PV tail is short.

P^T for the P@V matmul is produced by DMA xbar transposes (off the
tensor engine).
"""

import numpy as np
import ml_dtypes

import concourse.bass as bass
import concourse.mybir as mybir
import concourse.tile as tile
from concourse import bacc
from concourse.bass_utils import run_bass_kernel_spmd
from contextlib import ExitStack

B, S, D, DA = 4, 2048, 1024, 1024
NCORES = 8
F32 = mybir.dt.float32
BF16 = mybir.dt.bfloat16

ABLK = [0, 3, 4, 7, 8, 11, 12, 15]
BBLK = [1, 2, 5, 6, 9, 10, 13, 14]
NEG = -1e30

OWN_W = [min(l + 2, 8) for l in range(8)]  # own-run width (128-blocks)
OTH_W = [l + 1 for l in range(8)]          # other-run width


def _strip_positions(l):
    """strip block index -> permuted 128-block position (= V tile index)."""
    return list(range(OWN_W[l])) + [8 + k for k in range(OTH_W[l])]


def _chunks(l):
    """S-matmul chunks: (xt_col_start, strip_col_start, width<=512)."""
    out = []
    ow = OWN_W[l] * 128
    for st in range(0, ow, 512):
        out.append((st, st, min(512, ow - st)))
    tw = OTH_W[l] * 128
    for st in range(0, tw, 512):
        out.append((1024 + st, ow + st, min(512, tw - st)))
    return out


_CACHE = {}


def _build():
    if "nc" in _CACHE:
        return _CACHE["nc"]

    nc = bacc.Bacc()
    xth_d = nc.dram_tensor("xth", [D, S], BF16, kind="ExternalInput")
    xtl_d = nc.dram_tensor("xtl", [D, S], BF16, kind="ExternalInput")
    wqh_d = nc.dram_tensor("wqh", [DA, D], BF16, kind="ExternalInput")
    wql_d = nc.dram_tensor("wql", [DA, D], BF16, kind="ExternalInput")
    wkh_d = nc.dram_tensor("wkh", [DA, D], BF16, kind="ExternalInput")
    wkl_d = nc.dram_tensor("wkl", [DA, D], BF16, kind="ExternalInput")
    wvb_d = nc.dram_tensor("wvb", [D, DA], BF16, kind="ExternalInput")
    mka_d = nc.dram_tensor("mka", [1024, 256], BF16, kind="ExternalInput")
    mkb_d = nc.dram_tensor("mkb", [1024, 128], BF16, kind="ExternalInput")
    out_d = nc.dram_tensor("out", [1024, DA], F32, kind="ExternalOutput")

    with tile.TileContext(nc) as tc, ExitStack() as stack:
        # long-lived residents
        xpool = stack.enter_context(tc.tile_pool(name="xres", bufs=1))
        # xth[g][p, dp*512 + c] = bf16(x^T[dp*128+p, g*512+c]); xtl the residual
        xth = [xpool.tile([128, 4096], BF16, name=f"xth{g}", tag=f"xth{g}") for g in range(4)]
        xtl = [xpool.tile([128, 4096], BF16, name=f"xtl{g}", tag=f"xtl{g}") for g in range(4)]
        vpool = stack.enter_context(tc.tile_pool(name="vres", bufs=1))
        V = [vpool.tile([128, DA], BF16, name=f"v{p}", tag=f"v{p}") for p in range(16)]
        gpool = stack.enter_context(tc.tile_pool(name="gres", bufs=1))
        # G[d1, d2] tiles [d1-part][128, 1024] (hi/lo)
        Gh = [gpool.tile([128, 1024], BF16, name=f"gh{d}", tag=f"gh{d}") for d in range(8)]
        Gl = [gpool.tile([128, 1024], BF16, name=f"gl{d}", tag=f"gl{d}") for d in range(8)]
        # ---- input DMA (spread across queues; emission order = queue order)
        # gpsimd: wv, then x hi (V-phase order), then x lo
        # sync:   W hi/lo (needed ~60us in), later out stores
        # scalar: masks + PT transposes later
        with tc.tile_pool(name="wvp", bufs=1) as pwv:
            wvb = [pwv.tile([128, DA], BF16, name=f"wvb{d}", tag=f"wvb{d}") for d in range(8)]
            for dp in range(8):
                nc.gpsimd.dma_start(wvb[dp][:], wvb_d[dp * 128 : (dp + 1) * 128, :])
            for g in range(4):
                for dp in range(8):
                    nc.gpsimd.dma_start(
                        xth[g][:, dp * 512 : (dp + 1) * 512],
                        xth_d[dp * 128 : (dp + 1) * 128, g * 512 : (g + 1) * 512],
                    )
            for g in range(4):
                for dp in range(8):
                    nc.gpsimd.dma_start(
                        xtl[g][:, dp * 512 : (dp + 1) * 512],
                        xtl_d[dp * 128 : (dp + 1) * 128, g * 512 : (g + 1) * 512],
                    )
            # ---- Phase V: V = x @ Wv (single-term bf16) -------------------
            with tc.tile_pool(name="psV", bufs=4, space="PSUM") as psV:
                for g in range(4):
                    for q in range(4):
                        p = g * 4 + q
                        for half in range(2):
                            ps = psV.tile([128, 512], F32, tag="psv")
                            for dp in range(8):
                                nc.tensor.matmul(
                                    ps[:],
                                    xth[g][:, dp * 512 + q * 128 : dp * 512 + (q + 1) * 128],
                                    wvb[dp][:, half * 512 : (half + 1) * 512],
                                    start=(dp == 0),
                                    stop=(dp == 7),
                                )
                            nc.vector.tensor_copy(V[p][:, half * 512 : (half + 1) * 512], ps[:])

        # ---- Phase G: G = Wq Wk^T via 3-term split-bf16 -------------------
        # (W pool opens after wvp closes so SBUF fits; the W loads are the
        # first sync-queue work and so still execute from t~0)
        with tc.tile_pool(name="wqk", bufs=1) as pw, \
             tc.tile_pool(name="psG", bufs=3, space="PSUM") as psG:
            wqth = [pw.tile([128, D], BF16, name=f"wqth{a}", tag=f"wqth{a}") for a in range(8)]
            wqtl = [pw.tile([128, D], BF16, name=f"wqtl{a}", tag=f"wqtl{a}") for a in range(8)]
            wkth = [pw.tile([128, D], BF16, name=f"wkth{a}", tag=f"wkth{a}") for a in range(8)]
            wktl = [pw.tile([128, D], BF16, name=f"wktl{a}", tag=f"wktl{a}") for a in range(8)]
            for ap in range(8):
                rsl = slice(ap * 128, (ap + 1) * 128)
                nc.sync.dma_start(wqth[ap][:], wqh_d[rsl, :])
                nc.sync.dma_start(wqtl[ap][:], wql_d[rsl, :])
                nc.sync.dma_start(wkth[ap][:], wkh_d[rsl, :])
                nc.sync.dma_start(wktl[ap][:], wkl_d[rsl, :])
            for d1 in range(8):
                d1sl = slice(d1 * 128, (d1 + 1) * 128)
                for half in range(2):
                    hsl = slice(half * 512, (half + 1) * 512)
                    ps = psG.tile([128, 512], F32, tag="psg")
                    for ap in range(8):
                        nc.tensor.matmul(ps[:], wqth[ap][:, d1sl], wkth[ap][:, hsl], start=(ap == 0), stop=False)
                        nc.tensor.matmul(ps[:], wqth[ap][:, d1sl], wktl[ap][:, hsl], start=False, stop=False)
                        nc.tensor.matmul(ps[:], wqtl[ap][:, d1sl], wkth[ap][:, hsl], start=False, stop=(ap == 7))
                    nc.vector.tensor_copy(Gh[d1][:, hsl], ps[:])
                    nc.vector.tensor_sub(Gl[d1][:, hsl], ps[:], Gh[d1][:, hsl])

        # T^T[d, i] for own rows i (hi/lo): tiles [d-part][128, 1024]
        # (allocated after the W pool closes so addresses are reused)
        tpool = stack.enter_context(tc.tile_pool(name="tres", bufs=1))
        Th = [tpool.tile([128, 1024], BF16, name=f"th{d}", tag=f"th{d}") for d in range(8)]
        Tl = [tpool.tile([128, 1024], BF16, name=f"tl{d}", tag=f"tl{d}") for d in range(8)]

        # ---- Phase T: T^T = (x_own G)^T, own rows = groups 0,1 ------------
        with tc.tile_pool(name="psT", bufs=3, space="PSUM") as psT:
            for dpp in range(8):
                dsl = slice(dpp * 128, (dpp + 1) * 128)
                for half in range(2):
                    hsl = slice(half * 512, (half + 1) * 512)
                    ps = psT.tile([128, 512], F32, tag="pst")
                    for dp in range(8):
                        rh = xth[half][:, dp * 512 : (dp + 1) * 512]
                        rl = xtl[half][:, dp * 512 : (dp + 1) * 512]
                        nc.tensor.matmul(ps[:], Gh[dp][:, dsl], rh, start=(dp == 0), stop=False)
                        nc.tensor.matmul(ps[:], Gh[dp][:, dsl], rl, start=False, stop=False)
                        nc.tensor.matmul(ps[:], Gl[dp][:, dsl], rh, start=False, stop=(dp == 7))
                    nc.vector.tensor_copy(Th[dpp][:, hsl], ps[:])
                    nc.vector.tensor_sub(Tl[dpp][:, hsl], ps[:], Th[dpp][:, hsl])

        # ---- Phase attn: per own row-block l, descending strip width ------
        with (
            tc.tile_pool(name="pa", bufs=2) as pa,
            tc.tile_pool(name="pa1", bufs=3) as pa1,
            tc.tile_pool(name="psS", bufs=3, space="PSUM") as psS,
            tc.tile_pool(name="psO", bufs=2, space="PSUM") as psO,
        ):
            state = {}

            def emit_S(l):
                W = OWN_W[l] + OTH_W[l]
                S_sb = pa.tile([128, 2048], F32, name="S_sb", tag="S")
                for (xc0, sc0, w) in _chunks(l):
                    g, off = divmod(xc0, 512)
                    ps = psS.tile([128, 512], F32, name="ps", tag="ps")
                    for dp in range(8):
                        lh = Th[dp][:, l * 128 : (l + 1) * 128]
                        ll = Tl[dp][:, l * 128 : (l + 1) * 128]
                        rh = xth[g][:, dp * 512 + off : dp * 512 + off + w]
                        rl = xtl[g][:, dp * 512 + off : dp * 512 + off + w]
                        nc.tensor.matmul(ps[:, :w], lh, rh, start=(dp == 0), stop=False)
                        nc.tensor.matmul(ps[:, :w], lh, rl, start=False, stop=False)
                        nc.tensor.matmul(ps[:, :w], ll, rh, start=False, stop=(dp == 7))
                    nc.vector.tensor_copy(S_sb[:, sc0 : sc0 + w], ps[:, :w])
                mka = pa1.tile([128, 256], BF16, name="mka", tag="mka")
                nc.scalar.dma_start(mka[:], mka_d[l * 128 : (l + 1) * 128, :])
                w1 = slice(l * 128, (l + 2) * 128)
                nc.vector.tensor_add(S_sb[:, w1], S_sb[:, w1], mka[:])
                mkb = pa1.tile([128, 128], BF16, name="mkb", tag="mkb")
                nc.scalar.dma_start(mkb[:], mkb_d[l * 128 : (l + 1) * 128, :])
                w2 = slice((W - 1) * 128, W * 128)
                nc.vector.tensor_add(S_sb[:, w2], S_sb[:, w2], mkb[:])

                mx = pa1.tile([128, 1], F32, name="mx", tag="mx")
                nc.vector.reduce_max(mx[:], S_sb[:, : W * 128], axis=mybir.AxisListType.X)
                negb = pa1.tile([128, 1], F32, name="negb", tag="negb")
                nc.vector.tensor_scalar_mul(negb[:], mx[:], -1.0 / 32.0)
                P_sb = pa.tile([128, 2048], BF16, name="P_sb", tag="P")
                rs = pa1.tile([128, 1], F32, name="rs", tag="rs")
                nc.scalar.activation(
                    P_sb[:, : W * 128],
                    S_sb[:, : W * 128],
                    mybir.ActivationFunctionType.Exp,
                    bias=negb[:],
                    scale=1.0 / 32.0,
                    accum_out=rs[:],
                )
                PT = pa.tile([128, 2048], BF16, name="PT", tag="PT")
                eng = [nc.sync, nc.scalar]
                for b in range(W):
                    bsl = slice(b * 128, (b + 1) * 128)
                    eng[b % 2].dma_start_transpose(PT[:, bsl], P_sb[:, bsl])
                state[l] = (W, PT, rs)

            def emit_PV(l):
                W, PT, rs = state.pop(l)
                pos = _strip_positions(l)
                oacc = [psO.tile([128, 512], F32, name=f"oacc{h}", tag=f"oacc{h}") for h in range(2)]
                for b in range(W):
                    vj = pos[b]
                    bsl = slice(b * 128, (b + 1) * 128)
                    for half in range(2):
                        nc.tensor.matmul(
                            oacc[half][:],
                            PT[:, bsl],
                            V[vj][:, half * 512 : (half + 1) * 512],
                            start=(b == 0),
                            stop=(b == W - 1),
                        )
                rec = pa1.tile([128, 1], F32, name="rec", tag="rec")
                nc.vector.reciprocal(rec[:], rs[:])
                for half in range(2):
                    o_sb = pa1.tile([128, 512], F32, name=f"o{half}", tag=f"o{half}")
                    nc.vector.tensor_scalar_mul(o_sb[:], oacc[half][:], rec[:])
                    nc.sync.dma_start(
                        out_d[l * 128 : (l + 1) * 128, half * 512 : (half + 1) * 512],
                        o_sb[:],
                    )

            order = list(range(7, -1, -1))  # descending strip width
            for i, l in enumerate(order):
                emit_S(l)
                if i >= 1:
                    emit_PV(order[i - 1])
            emit_PV(order[-1])

    nc.compile()
    _CACHE["nc"] = nc
    return nc


def _split_bf16(a):
    hi = a.astype(ml_dtypes.bfloat16)
    lo = (a - hi.astype(np.float32)).astype(ml_dtypes.bfloat16)
    return np.ascontiguousarray(hi), np.ascontiguousarray(lo)


def _core_inputs(x, Wq, Wk, Wv, c):
    b = c // 2
    my = ABLK if c % 2 == 0 else BBLK
    oth = BBLK if c % 2 == 0 else ABLK
    permrows = np.concatenate([np.arange(g * 128, (g + 1) * 128) for g in my + oth])
    xt = np.ascontiguousarray(x[b][permrows].T)
    xth, xtl = _split_bf16(xt)
    wqh, wql = _split_bf16(np.ascontiguousarray(Wq.T))
    wkh, wkl = _split_bf16(np.ascontiguousarray(Wk.T))

    mka = np.zeros((1024, 256), dtype=ml_dtypes.bfloat16)
    mkb = np.zeros((1024, 128), dtype=ml_dtypes.bfloat16)
    for l in range(8):
        gi = my[l] * 128 + np.arange(128)
        strip = [my[k] for k in range(OWN_W[l])] + [oth[k] for k in range(OTH_W[l])]
        W = len(strip)
        for t, blk in enumerate((strip[l], strip[l + 1])):
            gj = blk * 128 + np.arange(128)
            mka[l * 128 : (l + 1) * 128, t * 128 : (t + 1) * 128] = np.where(
                gj[None, :] <= gi[:, None] + 1, 0.0, NEG
            )
        blk = strip[W - 1]
        gj = blk * 128 + np.arange(128)
        mkb[l * 128 : (l + 1) * 128, :] = np.where(gj[None, :] <= gi[:, None] + 1, 0.0, NEG)
        for p2, blk2 in enumerate(strip):
            if p2 in (l, l + 1, W - 1):
                continue
            assert blk2 * 128 + 127 <= my[l] * 128 + 1, (l, p2, blk2)

    return {
        "xth": xth,
        "xtl": xtl,
        "wqh": wqh,
        "wql": wql,
        "wkh": wkh,
        "wkl": wkl,
        "wvb": Wv.astype(ml_dtypes.bfloat16),
        "mka": mka,
        "mkb": mkb,
    }, (b, my)


def kernel(x, Wq, Wk, Wv):
    x = np.ascontiguousarray(np.asarray(x, dtype=np.float32))
    Wq = np.ascontiguousarray(np.asarray(Wq, dtype=np.float32))
    Wk = np.ascontiguousarray(np.asarray(Wk, dtype=np.float32))
    Wv = np.ascontiguousarray(np.asarray(Wv, dtype=np.float32))

    nc = _build()

    in_maps = []
    metas = []
    for c in range(NCORES):
        m, meta = _core_inputs(x, Wq, Wk, Wv, c)
        in_maps.append(m)
        metas.append(meta)

    res = run_bass_kernel_spmd(nc, in_maps, list(range(NCORES)))

    out = np.empty((B, S, DA), dtype=np.float32)
    for c in range(NCORES):
        b, my = metas[c]
        o = res.results[c]["out"]
        for l, g in enumerate(my):
            out[b, g * 128 : (g + 1) * 128] = o[l * 128 : (l + 1) * 128]
    return out
